# revision 6
# baseline (speedup 1.0000x reference)
"""AttentionPooler Trainium2 kernel (8-core SPMD).

Reference computation (per span s over layer-2 hidden states):
    spans = hs[idx_s, a_s:a_s+64, :]                      (64, 1024)
    proj  = spans @ W + b                                 (64, 256)
    scores= proj @ v, masked softmax over valid tokens
    out   = att @ proj                                    (256,)

Algebraic restructuring used here (exact math, no approximation):
    u = W @ v
    scores_s[l] = hs[n, l, :] @ u + (b@v)   -- the constant shifts all scores
                                               equally, softmax-invariant: drop it
    att  = softmax over the span window of g[n] = hs[n] @ u
    out  = (att @ hs[n, a:b]) @ W + b  =  att_full @ P[n] + b,  P[n] = hs[n] @ W

So instead of a per-span (64,1024)@(1024,256) projection (68.7 GFLOP total) we
project every position once per batch row (P = hs @ W, 1 GFLOP/core) and
per-span work collapses to a masked softmax plus a (128,512)@(512,256) matmul.

Sharding: batch rows N=32 are split 4-per-core; spans are routed on host to the
core/tile owning their batch row (spec sharding_hint option 2). Each core keeps
its 4 rows resident in SBUF in transposed layout (d-major, 8 MB), so the only
large HBM traffic is one read of the layer-2 shard.

Per-core device program (identical on all 8 cores, data differs):
  u      = W^T stripes @ v                                  (PE, tiny)
  P[n]   = hs[n] @ W        via lhsT=hsT chunks, rhs=W      (PE, 128 MM)
  scores = broadcast(u) @ hsT[n]   -> psum [128 spans, 512] (PE, 8 MM/tile)
  mask   = (iota>=a)*(iota<b)  per-partition span bounds    (DVE)
  att~   = exp(scores - rowmax) * mask, rowsum via fused ops (ACT+DVE)
  att^T  = PE transpose (4x 128x128)
  out    = (att^T.T @ P[n]) * (1/rowsum) + b                (PE + fused DVE)
"""

import numpy as np

import concourse.bass as bass
import concourse.tile as tile
from concourse import mybir
from concourse.bass_utils import run_bass_kernel_spmd

F32 = mybir.dt.float32
ALU = mybir.AluOpType
ACTF = mybir.ActivationFunctionType

LAYER = 2
N, L, D, H = 32, 512, 1024, 256
S, MAXSPAN = 2048, 64
NCORES = 8
NLOC = N // NCORES          # batch rows per core
CAP = 128                   # span slots per tile (one tile per local batch row)
DC = D // 128               # 8 contraction chunks over D
LC = L // 128               # 4 chunks over L
HCK = H // 128              # 2 chunks over H
BIG = 1e30


def _patch_tile_drain():
    """walrus in this container rejects >1 sync wait per CTRL instruction; the
    Tile kernel-tail drain accumulates one wait per live semaphore.  Split the
    waits across single-wait NOP carriers."""
    from concourse.mybir import SyncInfo
    from concourse.vector_clock import ScopedClock

    def _drain_and_barrier_split(self, tick_clock, wait_clock):
        nc = self.nc
        probe = nc.sync.nop()
        wait_clock.add_sem_waits(probe.ins, ScopedClock({None: tick_clock.global_clock}))
        si = probe.ins.sync_info
        waits = list(si.on_wait) if si is not None else []
        if len(waits) > 1:
            probe.ins.sync_info = SyncInfo(on_wait=waits[:1], on_update=list(si.on_update))
            for w in waits[1:]:
                carrier = nc.sync.nop()
                carrier.ins.sync_info = SyncInfo(on_wait=[w], on_update=[])
        nc.sync.drain()
        nc.all_engine_barrier()
        assert self.sems is not None
        popped = nc._tile_sem_poison_stack.pop()
        assert popped is self._sem_poison
        nc.clear_and_free_semaphores(list(self.sems.allocated().values()))
        nc.all_engine_barrier()

    tile.TileContext._drain_and_barrier = _drain_and_barrier_split


def _split_multi_waits(nc: bass.Bass) -> int:
    """walrus here allows at most ONE sync wait per instruction; Tile's sem
    assignment attaches one wait per cross-engine dependency.  Hoist the extra
    waits onto same-engine NoOp carriers inserted just before the instruction
    (sequential waits on monotonic semaphores are equivalent to a joint wait)."""
    from concourse.mybir import SyncInfo

    n_split = 0
    for f in nc.m.functions:
        for bb in f.blocks:
            insts = bb.instructions
            i = 0
            while i < len(insts):
                inst = insts[i]
                si = getattr(inst, "sync_info", None)
                waits = list(si.on_wait) if si is not None else []
                if len(waits) > 1:
                    inst.sync_info = SyncInfo(on_wait=[waits[-1]],
                                              on_update=list(si.on_update))
                    for w in waits[:-1]:
                        nop = mybir.InstNoOp(name=f"waitsplit-{nc.next_id()}",
                                             ins=[], outs=[])
                        nop.engine = inst.engine
                        nop.sync_info = SyncInfo(on_wait=[w], on_update=[])
                        nc.register_instruction(nop)
                        insts.insert(i, nop)
                        i += 1
                    n_split += 1
                i += 1
    return n_split


def build_program() -> bass.Bass:
    _patch_tile_drain()
    nc = bass.Bass()

    hsT_d = nc.dram_tensor("hsT", [NLOC, D, L], F32, kind="ExternalInput")
    w_d = nc.dram_tensor("w", [D, H], F32, kind="ExternalInput")
    wT_d = nc.dram_tensor("wT", [H, D], F32, kind="ExternalInput")
    vcol_d = nc.dram_tensor("vcol", [128, HCK], F32, kind="ExternalInput")
    acol_d = nc.dram_tensor("acol", [128, NLOC], F32, kind="ExternalInput")
    bcol_d = nc.dram_tensor("bcol", [128, NLOC], F32, kind="ExternalInput")
    bias_d = nc.dram_tensor("bias", [1, H], F32, kind="ExternalInput")
    iota_d = nc.dram_tensor("iota", [1, L], F32, kind="ExternalInput")
    ident_d = nc.dram_tensor("ident", [128, 128], F32, kind="ExternalInput")
    out_d = nc.dram_tensor("out", [NLOC, CAP, H], F32, kind="ExternalOutput")

    with tile.TileContext(nc) as tc:
        with (
            tc.tile_pool(name="persist", bufs=1) as PP,
            tc.tile_pool(name="work", bufs=2) as WK,
            tc.tile_pool(name="ps_sc", bufs=2, space="PSUM") as PS_SC,
            tc.tile_pool(name="ps_at", bufs=2, space="PSUM") as PS_AT,
            tc.tile_pool(name="ps_sm", bufs=3, space="PSUM") as PS_SM,
        ):
            # ---- constant / parameter loads ----
            w_sb = [PP.tile([128, H], F32, tag=f"w{dc}", name=f"w{dc}") for dc in range(DC)]
            for dc in range(DC):
                nc.sync.dma_start(out=w_sb[dc][:], in_=w_d[dc * 128:(dc + 1) * 128, :])
            wT_sb = [PP.tile([128, D], F32, tag=f"wT{hc}", name=f"wT{hc}") for hc in range(HCK)]
            for hc in range(HCK):
                nc.sync.dma_start(out=wT_sb[hc][:], in_=wT_d[hc * 128:(hc + 1) * 128, :])
            vcol_sb = PP.tile([128, HCK], F32, tag="vcol", name="vcol")
            nc.sync.dma_start(out=vcol_sb[:], in_=vcol_d[:])
            acol_sb = PP.tile([128, NLOC], F32, tag="acol", name="acol")
            nc.sync.dma_start(out=acol_sb[:], in_=acol_d[:])
            bcol_sb = PP.tile([128, NLOC], F32, tag="bcol", name="bcol")
            nc.sync.dma_start(out=bcol_sb[:], in_=bcol_d[:])
            bias_sb = PP.tile([128, H], F32, tag="bias", name="bias")
            nc.sync.dma_start(out=bias_sb[:], in_=bias_d[:].to_broadcast([128, H]))
            iota_sb = PP.tile([128, L], F32, tag="iota", name="iota")
            nc.sync.dma_start(out=iota_sb[:], in_=iota_d[:].to_broadcast([128, L]))
            ident_sb = PP.tile([128, 128], F32, tag="ident", name="ident")
            nc.sync.dma_start(out=ident_sb[:], in_=ident_d[:])
            ones_sb = PP.tile([128, 128], F32, tag="ones", name="ones")
            nc.vector.memset(ones_sb[:], 1.0)

            # ---- hidden-state shard (transposed layout), 8 MB resident ----
            hsT_sb = [[PP.tile([128, L], F32, tag=f"hsT{n}_{dc}", name=f"hsT{n}_{dc}") for dc in range(DC)]
                      for n in range(NLOC)]
            for n in range(NLOC):
                for dc in range(DC):
                    nc.sync.dma_start(out=hsT_sb[n][dc][:],
                                      in_=hsT_d[n, dc * 128:(dc + 1) * 128, :])

            # ---- u = W @ v  (via W^T stripes), broadcast along free dim ----
            ucol_sb = PP.tile([128, DC], F32, tag="ucol", name="ucol")
            for dt in range(DC):
                u_ps = PS_SM.tile([128, 1], F32, tag="ps_small", name="ps_small")
                for hc in range(HCK):
                    nc.tensor.matmul(u_ps[:], lhsT=wT_sb[hc][:, dt * 128:(dt + 1) * 128],
                                     rhs=vcol_sb[:, hc:hc + 1],
                                     start=(hc == 0), stop=(hc == HCK - 1))
                nc.scalar.copy(ucol_sb[:, dt:dt + 1], u_ps[:])
            ubc_sb = [PP.tile([128, 128], F32, tag=f"ubc{dc}", name=f"ubc{dc}") for dc in range(DC)]
            for dc in range(DC):
                nc.vector.tensor_scalar_mul(ubc_sb[dc][:], ones_sb[:],
                                            ucol_sb[:, dc:dc + 1])

            # ---- P[n] = hs[n] @ W for all 512 positions of each local row ----
            P_sb = [[PP.tile([128, H], F32, tag=f"P{n}_{lc}", name=f"P{n}_{lc}") for lc in range(LC)]
                    for n in range(NLOC)]
            for n in range(NLOC):
                for lc in range(LC):
                    p_ps = PS_SM.tile([128, H], F32, tag="ps_small", name="ps_small")
                    for dc in range(DC):
                        nc.tensor.matmul(p_ps[:],
                                         lhsT=hsT_sb[n][dc][:, lc * 128:(lc + 1) * 128],
                                         rhs=w_sb[dc][:],
                                         start=(dc == 0), stop=(dc == DC - 1))
                    nc.scalar.copy(P_sb[n][lc][:], p_ps[:])

            # ---- per span-tile: scores -> masked softmax -> pooled ----
            for t in range(NLOC):
                # scores[s, l] = g[n=t][l] for every span slot s (u broadcast)
                s_ps = PS_SC.tile([128, L], F32, tag="ps_scores", name="ps_scores")
                for dc in range(DC):
                    nc.tensor.matmul(s_ps[:], lhsT=ubc_sb[dc][:], rhs=hsT_sb[t][dc][:],
                                     start=(dc == 0), stop=(dc == DC - 1))

                # valid-token mask from per-span [a, b) bounds
                mlt = WK.tile([128, L], F32, tag="mlt", name="mlt")
                nc.vector.tensor_scalar(out=mlt[:], in0=iota_sb[:],
                                        scalar1=bcol_sb[:, t:t + 1], scalar2=None,
                                        op0=ALU.is_lt)
                mask = WK.tile([128, L], F32, tag="mask", name="mask")
                nc.vector.scalar_tensor_tensor(out=mask[:], in0=iota_sb[:],
                                               scalar=acol_sb[:, t:t + 1], in1=mlt[:],
                                               op0=ALU.is_ge, op1=ALU.mult)

                # exp(scores - rowmax); invalid positions masked to 0; row sums
                mxn = WK.tile([128, 1], F32, tag="mxn", name="mxn")
                nc.vector.reduce_max(out=mxn[:], in_=s_ps[:],
                                     axis=mybir.AxisListType.X, negate=True)
                ex = WK.tile([128, L], F32, tag="ex", name="ex")
                nc.scalar.activation(out=ex[:], in_=s_ps[:], func=ACTF.Exp,
                                     bias=mxn[:, 0:1], scale=1.0)
                am = WK.tile([128, L], F32, tag="am", name="am")
                ssum = WK.tile([128, 1], F32, tag="ssum", name="ssum")
                nc.vector.scalar_tensor_tensor(out=am[:], in0=ex[:], scalar=1.0,
                                               in1=mask[:], op0=ALU.mult,
                                               op1=ALU.mult,
                                               accum_out=ssum[:, 0:1])
                rec = WK.tile([128, 1], F32, tag="rec", name="rec")
                nc.vector.reciprocal(rec[:], ssum[:])

                # att^T via PE transpose, 4 x [128,128] into one psum bank
                at_ps = PS_AT.tile([128, L], F32, tag="ps_at", name="ps_at")
                for lc in range(LC):
                    nc.tensor.transpose(at_ps[:, lc * 128:(lc + 1) * 128],
                                        am[:, lc * 128:(lc + 1) * 128], ident_sb[:])
                at_sb = WK.tile([128, L], F32, tag="atsb", name="atsb")
                nc.vector.tensor_copy(at_sb[:], at_ps[:])

                # pooled = att @ P[t]   (contraction over position l)
                pl_ps = PS_SM.tile([128, H], F32, tag="ps_small", name="ps_small")
                for lc in range(LC):
                    nc.tensor.matmul(pl_ps[:], lhsT=at_sb[:, lc * 128:(lc + 1) * 128],
                                     rhs=P_sb[t][lc][:],
                                     start=(lc == 0), stop=(lc == LC - 1))
                o_sb = WK.tile([128, H], F32, tag="osb", name="osb")
                nc.vector.scalar_tensor_tensor(out=o_sb[:], in0=pl_ps[:],
                                               scalar=rec[:, 0:1], in1=bias_sb[:],
                                               op0=ALU.mult, op1=ALU.add)
                nc.sync.dma_start(out=out_d[t], in_=o_sb[:])

    _split_multi_waits(nc)
    return nc


def prepare_inputs(hidden_states, target_spans, W, b, v):
    """Host-side sharding/routing: slice the probed layer, transpose per-core
    shards to d-major, route spans to the core/tile owning their batch row."""
    hs2 = np.ascontiguousarray(np.asarray(hidden_states, dtype=np.float32)[LAYER])
    spans = np.asarray(target_spans)
    idx = spans[:, 0].astype(np.int64)
    a = spans[:, 1].astype(np.int64)
    bb = spans[:, 2].astype(np.int64)

    Wf = np.ascontiguousarray(np.asarray(W, dtype=np.float32))
    WTf = np.ascontiguousarray(Wf.T)
    vf = np.asarray(v, dtype=np.float32)
    bf = np.asarray(b, dtype=np.float32)
    vcol = np.ascontiguousarray(vf.reshape(HCK, 128).T)
    bias_row = np.ascontiguousarray(bf.reshape(1, H))
    iota_row = np.arange(L, dtype=np.float32).reshape(1, L)
    ident = np.eye(128, dtype=np.float32)

    in_maps = []
    scatter = []  # scatter[c][t] = global span ids in slot order
    for c in range(NCORES):
        hsT = np.ascontiguousarray(hs2[c * NLOC:(c + 1) * NLOC].transpose(0, 2, 1))
        acol = np.zeros((CAP, NLOC), dtype=np.float32)
        bcol = np.ones((CAP, NLOC), dtype=np.float32)  # pad spans -> [0, 1)
        core_scatter = []
        for t in range(NLOC):
            ids = np.nonzero(idx == c * NLOC + t)[0]
            if len(ids) > CAP:
                raise ValueError(
                    f"span tile overflow: batch row {c * NLOC + t} has {len(ids)} "
                    f"spans > capacity {CAP}")
            acol[:len(ids), t] = a[ids]
            bcol[:len(ids), t] = bb[ids]
            core_scatter.append(ids)
        in_maps.append({
            "hsT": hsT, "w": Wf, "wT": WTf, "vcol": vcol,
            "acol": acol, "bcol": bcol, "bias": bias_row,
            "iota": iota_row, "ident": ident,
        })
        scatter.append(core_scatter)
    return in_maps, scatter


_NC_CACHE = {}


def _get_nc():
    if "nc" not in _NC_CACHE:
        _NC_CACHE["nc"] = build_program()
    return _NC_CACHE["nc"]


def kernel(hidden_states, target_spans, W, b, v, _trace=False, **_trace_kwargs):
    nc = _get_nc()
    in_maps, scatter = prepare_inputs(hidden_states, target_spans, W, b, v)
    res = run_bass_kernel_spmd(nc, in_maps, list(range(NCORES)), trace=_trace,
                               **_trace_kwargs)
    kernel.last_result = res

    out = np.empty((S, H), dtype=np.float32)
    for c in range(NCORES):
        o = res.results[c]["out"]
        for t in range(NLOC):
            ids = scatter[c][t]
            out[ids] = o[t, :len(ids)]
    return out


kernel.last_result = None


# revision 9
# speedup vs baseline: 1.1778x; 1.1778x over previous
"""AttentionPooler Trainium2 kernel (8-core SPMD).

Reference computation (per span s over layer-2 hidden states):
    spans = hs[idx_s, a_s:a_s+64, :]                      (64, 1024)
    proj  = spans @ W + b                                 (64, 256)
    scores= proj @ v, masked softmax over valid tokens
    out   = att @ proj                                    (256,)

Algebraic restructuring used here (exact math, no approximation):
    u = W @ v
    scores_s[l] = hs[n, l, :] @ u + (b@v)   -- the constant shifts all scores
                                               equally, softmax-invariant: drop it
    att  = softmax over the span window of g[n] = hs[n] @ u
    out  = (att @ hs[n, a:b]) @ W + b  =  att_full @ P[n] + b,  P[n] = hs[n] @ W

So instead of a per-span (64,1024)@(1024,256) projection (68.7 GFLOP total) we
project every position once per batch row (P = hs @ W, 1 GFLOP/core) and
per-span work collapses to a masked softmax plus a (128,512)@(512,256) matmul.

Sharding: batch rows N=32 are split 4-per-core; spans are routed on host to the
core/tile owning their batch row (spec sharding_hint option 2). Each core keeps
its 4 rows resident in SBUF in transposed layout (d-major, 8 MB), so the only
large HBM traffic is one read of the layer-2 shard.

Per-core device program (identical on all 8 cores, data differs):
  u      = W^T stripes @ v                                  (PE, tiny)
  P[n]   = hs[n] @ W        via lhsT=hsT chunks, rhs=W      (PE, 128 MM)
  scores = broadcast(u) @ hsT[n]   -> psum [128 spans, 512] (PE, 8 MM/tile)
  mask   = (iota>=a)*(iota<b)  per-partition span bounds    (DVE)
  att~   = exp(scores - rowmax) * mask, rowsum via fused ops (ACT+DVE)
  att^T  = PE transpose (4x 128x128)
  out    = (att^T.T @ P[n]) * (1/rowsum) + b                (PE + fused DVE)
"""

import numpy as np

import concourse.bass as bass
import concourse.tile as tile
from concourse import mybir
from concourse.bass_utils import run_bass_kernel_spmd

F32 = mybir.dt.float32
ALU = mybir.AluOpType
ACTF = mybir.ActivationFunctionType

LAYER = 2
N, L, D, H = 32, 512, 1024, 256
S, MAXSPAN = 2048, 64
NCORES = 8
NLOC = N // NCORES          # batch rows per core
CAP = 128                   # span slots per tile (one tile per local batch row)
DC = D // 128               # 8 contraction chunks over D
LC = L // 128               # 4 chunks over L
HCK = H // 128              # 2 chunks over H
BIG = 1e30


def _patch_tile_drain():
    """walrus in this container rejects >1 sync wait per CTRL instruction; the
    Tile kernel-tail drain accumulates one wait per live semaphore.  Split the
    waits across single-wait NOP carriers."""
    from concourse.mybir import SyncInfo
    from concourse.vector_clock import ScopedClock

    def _drain_and_barrier_split(self, tick_clock, wait_clock):
        nc = self.nc
        probe = nc.sync.nop()
        wait_clock.add_sem_waits(probe.ins, ScopedClock({None: tick_clock.global_clock}))
        si = probe.ins.sync_info
        waits = list(si.on_wait) if si is not None else []
        if len(waits) > 1:
            probe.ins.sync_info = SyncInfo(on_wait=waits[:1], on_update=list(si.on_update))
            for w in waits[1:]:
                carrier = nc.sync.nop()
                carrier.ins.sync_info = SyncInfo(on_wait=[w], on_update=[])
        nc.sync.drain()
        nc.all_engine_barrier()
        assert self.sems is not None
        popped = nc._tile_sem_poison_stack.pop()
        assert popped is self._sem_poison
        nc.clear_and_free_semaphores(list(self.sems.allocated().values()))
        nc.all_engine_barrier()

    tile.TileContext._drain_and_barrier = _drain_and_barrier_split


def _split_multi_waits(nc: bass.Bass) -> int:
    """walrus here allows at most ONE sync wait per instruction; Tile's sem
    assignment attaches one wait per cross-engine dependency.  Hoist the extra
    waits onto same-engine NoOp carriers inserted just before the instruction
    (sequential waits on monotonic semaphores are equivalent to a joint wait)."""
    from concourse.mybir import SyncInfo

    n_split = 0
    for f in nc.m.functions:
        for bb in f.blocks:
            insts = bb.instructions
            i = 0
            while i < len(insts):
                inst = insts[i]
                si = getattr(inst, "sync_info", None)
                waits = list(si.on_wait) if si is not None else []
                if len(waits) > 1:
                    inst.sync_info = SyncInfo(on_wait=[waits[-1]],
                                              on_update=list(si.on_update))
                    for w in waits[:-1]:
                        nop = mybir.InstNoOp(name=f"waitsplit-{nc.next_id()}",
                                             ins=[], outs=[])
                        nop.engine = inst.engine
                        nop.sync_info = SyncInfo(on_wait=[w], on_update=[])
                        nc.register_instruction(nop)
                        insts.insert(i, nop)
                        i += 1
                    n_split += 1
                i += 1
    return n_split


def build_program() -> bass.Bass:
    _patch_tile_drain()
    nc = bass.Bass()

    hsT_d = nc.dram_tensor("hsT", [NLOC, D, L], F32, kind="ExternalInput")
    w_d = nc.dram_tensor("w", [D, H], F32, kind="ExternalInput")
    wT_d = nc.dram_tensor("wT", [H, D], F32, kind="ExternalInput")
    vcol_d = nc.dram_tensor("vcol", [128, HCK], F32, kind="ExternalInput")
    acol_d = nc.dram_tensor("acol", [128, NLOC], F32, kind="ExternalInput")
    bcol_d = nc.dram_tensor("bcol", [128, NLOC], F32, kind="ExternalInput")
    bias_d = nc.dram_tensor("bias", [1, H], F32, kind="ExternalInput")
    iota_d = nc.dram_tensor("iota", [1, L], F32, kind="ExternalInput")
    ident_d = nc.dram_tensor("ident", [128, 128], F32, kind="ExternalInput")
    out_d = nc.dram_tensor("out", [NLOC, CAP, H], F32, kind="ExternalOutput")

    with tile.TileContext(nc) as tc:
        with (
            tc.tile_pool(name="persist", bufs=1) as PP,
            tc.tile_pool(name="work", bufs=2) as WK,
            tc.tile_pool(name="ps_p", bufs=3, space="PSUM") as PS_P,
            tc.tile_pool(name="ps_at", bufs=2, space="PSUM") as PS_AT,
            tc.tile_pool(name="ps_sm", bufs=2, space="PSUM") as PS_SM,
            tc.tile_pool(name="dram", bufs=1, space="DRAM") as DP,
        ):
            # ---- parameter / constant loads (order matters: u-path first) ----
            wT_sb = [PP.tile([128, D], F32, tag=f"wT{hc}", name=f"wT{hc}") for hc in range(HCK)]
            for hc in range(HCK):
                nc.sync.dma_start(out=wT_sb[hc][:], in_=wT_d[hc * 128:(hc + 1) * 128, :])
            vcol_sb = PP.tile([128, HCK], F32, tag="vcol", name="vcol")
            nc.sync.dma_start(out=vcol_sb[:], in_=vcol_d[:])
            # W' = [W | u]: column H holds u = W @ v so one matmul produces the
            # projection AND the per-position attention logits g = hs @ u.
            w_sb = [PP.tile([128, H + 1], F32, tag=f"w{dc}", name=f"w{dc}") for dc in range(DC)]
            for dc in range(DC):
                nc.sync.dma_start(out=w_sb[dc][:, 0:H], in_=w_d[dc * 128:(dc + 1) * 128, :])
            acol_sb = PP.tile([128, NLOC], F32, tag="acol", name="acol")
            nc.sync.dma_start(out=acol_sb[:], in_=acol_d[:])
            bcol_sb = PP.tile([128, NLOC], F32, tag="bcol", name="bcol")
            nc.sync.dma_start(out=bcol_sb[:], in_=bcol_d[:])
            bias_sb = PP.tile([128, H], F32, tag="bias", name="bias")
            nc.sync.dma_start(out=bias_sb[:], in_=bias_d[:].to_broadcast([128, H]))
            iota_sb = PP.tile([128, L], F32, tag="iota", name="iota")
            nc.sync.dma_start(out=iota_sb[:], in_=iota_d[:].to_broadcast([128, L]))
            ident_sb = PP.tile([128, 128], F32, tag="ident", name="ident")
            nc.sync.dma_start(out=ident_sb[:], in_=ident_d[:])

            # ---- hidden-state shard (transposed layout), 8 MB resident ----
            hsT_sb = [[PP.tile([128, L], F32, tag=f"hsT{n}_{dc}", name=f"hsT{n}_{dc}") for dc in range(DC)]
                      for n in range(NLOC)]
            for n in range(NLOC):
                for dc in range(DC):
                    nc.sync.dma_start(out=hsT_sb[n][dc][:],
                                      in_=hsT_d[n, dc * 128:(dc + 1) * 128, :])

            # ---- u = W @ v (via W^T stripes) -> column H of W' ----
            for dt in range(DC):
                u_ps = PS_SM.tile([128, 1], F32, tag="ps_small", name="ps_small")
                for hc in range(HCK):
                    nc.tensor.matmul(u_ps[:], lhsT=wT_sb[hc][:, dt * 128:(dt + 1) * 128],
                                     rhs=vcol_sb[:, hc:hc + 1],
                                     start=(hc == 0), stop=(hc == HCK - 1))
                nc.vector.tensor_copy(w_sb[dt][:, H:H + 1], u_ps[:])

            g4all_sb = PP.tile([128, NLOC * LC], F32, tag="g4all", name="g4all")
            g_row_dr = DP.tile([NLOC, L], F32, tag="g_row", name="g_row")
            P_sb = [[PP.tile([128, H], F32, tag=f"P{n}_{lc}", name=f"P{n}_{lc}") for lc in range(LC)]
                    for n in range(NLOC)]

            def emit_proj(n):
                # P'[n] = hs[n] @ [W | u]; split evac: P body -> P_sb, g col -> g4all
                for lc in range(LC):
                    p_ps = PS_P.tile([128, H + 1], F32, tag="ps_proj", name="ps_proj")
                    for dc in range(DC):
                        nc.tensor.matmul(p_ps[:],
                                         lhsT=hsT_sb[n][dc][:, lc * 128:(lc + 1) * 128],
                                         rhs=w_sb[dc][:],
                                         start=(dc == 0), stop=(dc == DC - 1))
                    nc.scalar.copy(P_sb[n][lc][:], p_ps[:, 0:H])
                    nc.vector.tensor_copy(g4all_sb[:, n * LC + lc:n * LC + lc + 1],
                                          p_ps[:, H:H + 1])
                # g row of n -> one partition of g_row_sb (flatten via PE transpose + DMA)
                gt_ps = PS_SM.tile([LC, 128], F32, tag="ps_small", name="ps_small")
                nc.tensor.transpose(gt_ps[:], g4all_sb[:, n * LC:(n + 1) * LC], ident_sb[:])
                gt_sb = WK.tile([LC, 128], F32, tag="gt_sb", name="gt_sb")
                nc.vector.tensor_copy(gt_sb[:], gt_ps[:])
                nc.gpsimd.dma_start(out=g_row_dr[n:n + 1, :], in_=gt_sb[:])

            def emit_tile(t):
                # scores[s, l] = g[t][l] replicated to all span slots via DMA bcast
                scores = WK.tile([128, L], F32, tag="scores", name="scores")
                nc.gpsimd.dma_start(out=scores[:],
                                    in_=g_row_dr[t:t + 1, :].to_broadcast([128, L]))
                # valid-token mask from per-span [a, b) bounds
                mlt = WK.tile([128, L], F32, tag="mlt", name="mlt")
                nc.vector.tensor_scalar(out=mlt[:], in0=iota_sb[:],
                                        scalar1=bcol_sb[:, t:t + 1], scalar2=None,
                                        op0=ALU.is_lt)
                mask = WK.tile([128, L], F32, tag="mask", name="mask")
                nc.vector.scalar_tensor_tensor(out=mask[:], in0=iota_sb[:],
                                               scalar=acol_sb[:, t:t + 1], in1=mlt[:],
                                               op0=ALU.is_ge, op1=ALU.mult)
                # exp(scores - rowmax) * mask, with fused row sums
                mxn = WK.tile([128, 1], F32, tag="mxn", name="mxn")
                nc.vector.reduce_max(out=mxn[:], in_=scores[:],
                                     axis=mybir.AxisListType.X, negate=True)
                ex = WK.tile([128, L], F32, tag="ex", name="ex")
                nc.scalar.activation(out=ex[:], in_=scores[:], func=ACTF.Exp,
                                     bias=mxn[:, 0:1], scale=1.0)
                am = WK.tile([128, L], F32, tag="am", name="am")
                ssum = WK.tile([128, 1], F32, tag="ssum", name="ssum")
                nc.vector.scalar_tensor_tensor(out=am[:], in0=ex[:], scalar=1.0,
                                               in1=mask[:], op0=ALU.mult,
                                               op1=ALU.mult,
                                               accum_out=ssum[:, 0:1])
                rec = WK.tile([128, 1], F32, tag="rec", name="rec")
                nc.vector.reciprocal(rec[:], ssum[:])
                # att^T via PE transpose, 4 x [128,128] into one psum bank
                at_ps = PS_AT.tile([128, L], F32, tag="ps_at", name="ps_at")
                for lc in range(LC):
                    nc.tensor.transpose(at_ps[:, lc * 128:(lc + 1) * 128],
                                        am[:, lc * 128:(lc + 1) * 128], ident_sb[:])
                at_sb = WK.tile([128, L], F32, tag="atsb", name="atsb")
                nc.vector.tensor_copy(at_sb[:], at_ps[:])
                # pooled = att @ P[t] (contraction over position l)
                pl_ps = PS_SM.tile([128, H], F32, tag="ps_small", name="ps_small")
                for lc in range(LC):
                    nc.tensor.matmul(pl_ps[:], lhsT=at_sb[:, lc * 128:(lc + 1) * 128],
                                     rhs=P_sb[t][lc][:],
                                     start=(lc == 0), stop=(lc == LC - 1))
                o_sb = WK.tile([128, H], F32, tag="osb", name="osb")
                nc.vector.scalar_tensor_tensor(out=o_sb[:], in0=pl_ps[:],
                                               scalar=rec[:, 0:1], in1=bias_sb[:],
                                               op0=ALU.mult, op1=ALU.add)
                nc.gpsimd.dma_start(out=out_d[t], in_=o_sb[:])

            # software-pipelined emission: tile(n-1) work overlaps P'(n) matmuls
            emit_proj(0)
            for n in range(1, NLOC):
                emit_proj(n)
                emit_tile(n - 1)
            emit_tile(NLOC - 1)

    _split_multi_waits(nc)
    return nc


def prepare_inputs(hidden_states, target_spans, W, b, v):
    """Host-side sharding/routing: slice the probed layer, transpose per-core
    shards to d-major, route spans to the core/tile owning their batch row."""
    hs2 = np.ascontiguousarray(np.asarray(hidden_states, dtype=np.float32)[LAYER])
    spans = np.asarray(target_spans)
    idx = spans[:, 0].astype(np.int64)
    a = spans[:, 1].astype(np.int64)
    bb = spans[:, 2].astype(np.int64)

    Wf = np.ascontiguousarray(np.asarray(W, dtype=np.float32))
    WTf = np.ascontiguousarray(Wf.T)
    vf = np.asarray(v, dtype=np.float32)
    bf = np.asarray(b, dtype=np.float32)
    vcol = np.ascontiguousarray(vf.reshape(HCK, 128).T)
    bias_row = np.ascontiguousarray(bf.reshape(1, H))
    iota_row = np.arange(L, dtype=np.float32).reshape(1, L)
    ident = np.eye(128, dtype=np.float32)

    in_maps = []
    scatter = []  # scatter[c][t] = global span ids in slot order
    for c in range(NCORES):
        hsT = np.ascontiguousarray(hs2[c * NLOC:(c + 1) * NLOC].transpose(0, 2, 1))
        acol = np.zeros((CAP, NLOC), dtype=np.float32)
        bcol = np.ones((CAP, NLOC), dtype=np.float32)  # pad spans -> [0, 1)
        core_scatter = []
        for t in range(NLOC):
            ids = np.nonzero(idx == c * NLOC + t)[0]
            if len(ids) > CAP:
                raise ValueError(
                    f"span tile overflow: batch row {c * NLOC + t} has {len(ids)} "
                    f"spans > capacity {CAP}")
            acol[:len(ids), t] = a[ids]
            bcol[:len(ids), t] = bb[ids]
            core_scatter.append(ids)
        in_maps.append({
            "hsT": hsT, "w": Wf, "wT": WTf, "vcol": vcol,
            "acol": acol, "bcol": bcol, "bias": bias_row,
            "iota": iota_row, "ident": ident,
        })
        scatter.append(core_scatter)
    return in_maps, scatter


_NC_CACHE = {}


def _get_nc():
    if "nc" not in _NC_CACHE:
        _NC_CACHE["nc"] = build_program()
    return _NC_CACHE["nc"]


def kernel(hidden_states, target_spans, W, b, v, _trace=False, **_trace_kwargs):
    nc = _get_nc()
    in_maps, scatter = prepare_inputs(hidden_states, target_spans, W, b, v)
    res = run_bass_kernel_spmd(nc, in_maps, list(range(NCORES)), trace=_trace,
                               **_trace_kwargs)
    kernel.last_result = res

    out = np.empty((S, H), dtype=np.float32)
    for c in range(NCORES):
        o = res.results[c]["out"]
        for t in range(NLOC):
            ids = scatter[c][t]
            out[ids] = o[t, :len(ids)]
    return out


kernel.last_result = None
